# revision 4
# baseline (speedup 1.0000x reference)
"""
Multi-head attention + residual + LayerNorm Trainium2 kernel (8 NeuronCores).
v3: fp8 with HW-calibrated matmul choices.

Shapes: hidden_states [2, 2048, 1024] f32, mask [2, 2048, 2048] int32,
Wq/Wk/Wv/Wd [1024, 1024] f32, bd/gamma/beta [1024] f32; 16 heads, hd=64.

Sharding: data parallel (core c: batch c//4, query rows (c%4)*512..+512);
K/V computed for the full sequence per core; no collectives.

Matmul mode choices (real-HW cost = ldweights(serial if same rows) + stream):
  - projections / dense: fp8 DoubleRow (2 MACs/cell halves the stream;
    ldweights 2*128 cols is the price, still the cheapest per k-tile)
  - scores: fp8 normal mode, K=64, natural head-pair layout. FWL (fast
    weight load, auto at 128 weight cols) + the two heads of a pair run on
    disjoint row-group halves of the PE array so their ldweights/streams
    overlap.
  - mask: every matmul pays a flat ~390 ns issue floor on this HW, so the
    mask is split by engine: even kv-tiles add D*I identity matmuls into
    the score PSUM before exp (-7.03 in exp units, leak e^-7 negligible);
    odd kv-tiles skip the PE entirely and multiply the fp8 probabilities
    by a {0,1} mask on the vector engine after exp (K_MDVE_TS knob).
  - ctx: fp8 DoubleRow over kv-tile pairs, v rows padded to 66 so the
    pair stride (8*66=528) hits the %16 ldweights constraint; a ones
    column in V row 64 yields the softmax normalizer for free.
Softmax exp: scalar-engine activation Exp (scale 1/8192) straight from
PSUM into fp8, except tiles listed in DVE_TS which use the DVE bit-trick
(u8 = sat(S*1.4427/1024 + 56) IS the fp8 byte pattern of e^(s/8));
engine split is a load-balancing knob.  Normalization batched at the end
(bf16 reciprocal + selector matmul broadcast + requantize to fp8).
Dense PSUM = 1024 * dense_true; host pre-scales the residual by 1024 and
LayerNorm's scale invariance absorbs it.
"""

import os
import sys
from collections import deque
from contextlib import ExitStack

import numpy as np

for _p in ("/opt/trn_rl_repo",):
    if os.path.isdir(_p) and _p not in sys.path:
        sys.path.insert(0, _p)

import ml_dtypes  # noqa: E402

import concourse.bass as bass  # noqa: E402
import concourse.tile as tile  # noqa: E402
from concourse import bacc, mybir  # noqa: E402
from concourse.bass_utils import run_bass_kernel_spmd  # noqa: E402

BF16 = mybir.dt.bfloat16
F32 = mybir.dt.float32
F8 = mybir.dt.float8e4
U8 = mybir.dt.uint8
NP_F8 = ml_dtypes.float8_e4m3  # IEEE e4m3: max finite 240, matches device
NP_BF16 = ml_dtypes.bfloat16

B, S, H, NH = 2, 2048, 1024, 16
HD = H // NH  # 64
P = 128
NCORES = 8
SQ = S // 4
FC = H // P          # 8 feature chunks
KC = S // P          # 16 kv tiles
NW = S // 512        # 4 kv windows
WSCALE = 32.0
D_MASK = 240.0
EXP_SCALE = 1.0 / (8.0 * WSCALE * WSCALE)
A_DVE = 8.0 * 1.4427 * EXP_SCALE
B_DVE = 56.0
EPS = 1e-6
DR = mybir.MatmulPerfMode.DoubleRow
SWI = mybir.MatmulPerfMode.DoubleRowSwInterleave
USE_SWI = os.environ.get("K_SWI", "0") == "1"

MASK_DMA = os.environ.get("K_MASK", "pe") == "dma"
DVE_TS = tuple(int(x) for x in
               os.environ.get("K_DVE_TS", "").split(",") if x != "")
MDVE_TS = tuple(int(x) for x in
                os.environ.get("K_MDVE_TS", "1,3,5,7,9,11,13,15").split(",") if x != "")

last_results = None


def _pair0(ap):
    """Insert a 0-stride pair dim after the partition dim: [K, n] -> [K, 2, n]."""
    return bass.AP(tensor=ap.tensor, offset=ap.offset,
                   ap=[ap.ap[0], [0, 2]] + list(ap.ap[1:]))


def _bcast_ap(src_1d, parts):
    return bass.AP(tensor=src_1d.tensor, offset=src_1d.offset,
                   ap=[[0, parts]] + list(src_1d.ap))


def _build_program(affine=True):
    nc = bacc.Bacc("TRN2", target_bir_lowering=False, debug=False,
                   enable_asserts=False, num_devices=NCORES)

    d_xT = nc.dram_tensor("xT", [FC, P, S], F8, kind="ExternalInput").ap()
    d_wq = nc.dram_tensor("wqT", [FC // 2, P, 2 * H], F8, kind="ExternalInput").ap()
    d_wk = nc.dram_tensor("wkT", [FC // 2, P, 2 * H], F8, kind="ExternalInput").ap()
    d_wv = nc.dram_tensor("wvT", [FC, P, H], F8, kind="ExternalInput").ap()
    d_wd = nc.dram_tensor("wdT", [FC, P, H], F8, kind="ExternalInput").ap()
    d_maskT = nc.dram_tensor("maskT", [KC, P, SQ], F8, kind="ExternalInput").ap()
    d_mask1 = nc.dram_tensor("mask1", [KC, P, SQ], F8, kind="ExternalInput").ap()
    d_identDD = nc.dram_tensor("identDD", [P, P], F8, kind="ExternalInput").ap()
    d_xres = nc.dram_tensor("xres", [SQ // P, P, H], F32, kind="ExternalInput").ap()
    d_gamma = nc.dram_tensor("gamma", [H], F32, kind="ExternalInput").ap()
    d_beta = nc.dram_tensor("beta", [H], F32, kind="ExternalInput").ap()
    d_sel = nc.dram_tensor("sel", [NH, FC, P], BF16, kind="ExternalInput").ap()
    d_out = nc.dram_tensor("out", [SQ // P, P, H], F32, kind="ExternalOutput").ap()

    with tile.TileContext(nc, trace_sim=False) as tc, \
            nc.allow_low_precision(reason="fp8 attention: error budget 2e-2"):
        with ExitStack() as ctx:
            _program(ctx, tc, d_xT, d_wq, d_wk, d_wv, d_wd, d_maskT, d_mask1,
                     d_identDD, d_xres, d_gamma, d_beta, d_sel, d_out, affine)

    nc.compile()
    return nc


def _program(ctx, tc, d_xT, d_wq, d_wk, d_wv, d_wd, d_maskT, d_mask1,
             d_identDD, d_xres, d_gamma, d_beta, d_sel, d_out, affine):
    nc = tc.nc

    persist = ctx.enter_context(tc.tile_pool(name="persist", bufs=1))
    ps_mm = ctx.enter_context(tc.tile_pool(name="ps_mm", bufs=2, space="PSUM"))
    ps_s = ctx.enter_context(tc.tile_pool(name="ps_s", bufs=2, space="PSUM"))
    ps_c = ctx.enter_context(tc.tile_pool(name="ps_c", bufs=2, space="PSUM"))

    kT_all = persist.tile([P, FC, S], F8, name="kT")     # head-pair rows
    qT_all = persist.tile([P, FC, SQ], F8, name="qT")
    v_g = [persist.tile([P, KC, 8, HD + 2], F8, name=f"v{g}") for g in range(2)]
    if MASK_DMA:
        maskT_sb = persist.tile([P, KC, 2, SQ], F8, name="maskT")
    else:
        maskT_sb = persist.tile([P, KC, SQ], F8, name="maskT")
    identDD = persist.tile([P, P], F8, name="identDD")
    mask1_sb = persist.tile([P, KC, SQ], F8, name="mask1")
    ctxT_sb = persist.tile([P, FC, SQ], BF16, name="ctxT")
    ctxT8 = persist.tile([P, FC, SQ], F8, name="ctxT8")
    sums2 = [persist.tile([8, SQ], F32, name=f"sums{i}") for i in range(2)]
    # 4-slot t-ring of masked probabilities; u8 bytes are fp8 values.
    # [partition, head-of-pair, t%4, q]
    pT_all = persist.tile([P, 2, 4, SQ], U8, name="pT")

    if MASK_DMA:
        nc.gpsimd.dma_start(out=maskT_sb[:, :, 0, :],
                            in_=d_maskT.rearrange("c p n -> p c n"))
        nc.gpsimd.dma_start(out=maskT_sb[:, :, 1, :],
                            in_=d_maskT.rearrange("c p n -> p c n"))
    else:
        nc.gpsimd.dma_start(out=maskT_sb, in_=d_maskT.rearrange("c p n -> p c n"))
    nc.gpsimd.dma_start(out=identDD, in_=d_identDD)
    nc.gpsimd.dma_start(out=mask1_sb, in_=d_mask1.rearrange("c p n -> p c n"))
    for g in range(2):
        nc.vector.memset(v_g[g][:, :, :, HD:HD + 1], 1.0)
        nc.vector.memset(v_g[g][:, :, :, HD + 1:HD + 2], 0.0)

    work = ctx.enter_context(tc.tile_pool(name="work", bufs=3))

    # ---------------- projection phase ----------------
    proj_ctx = ExitStack()
    pool_xt = proj_ctx.enter_context(tc.tile_pool(name="proj_xt", bufs=1))
    xT_sb = pool_xt.tile([P, FC, S], F8)
    pool_w = proj_ctx.enter_context(tc.tile_pool(name="proj_w", bufs=1))
    wq_sb = pool_w.tile([P, FC // 2, 2 * H], F8)
    wk_sb = pool_w.tile([P, FC // 2, 2 * H], F8)
    wv_sb = pool_w.tile([P, FC, H], F8)
    eng = [nc.sync, nc.scalar]
    nc.sync.dma_start(out=xT_sb[:, 0, :], in_=d_xT[0])
    nc.scalar.dma_start(out=xT_sb[:, 1, :], in_=d_xT[1])
    for half in range(2):
        cs = slice(half * 2, half * 2 + 2)
        eng[half].dma_start(out=wq_sb[:, cs, :],
                            in_=d_wq[cs].rearrange("c p n -> p c n"))
    for c in range(2, FC):
        eng[c % 2].dma_start(out=xT_sb[:, c, :], in_=d_xT[c])
    for half in range(2):
        cs = slice(half * 2, half * 2 + 2)
        eng[half].dma_start(out=wk_sb[:, cs, :],
                            in_=d_wk[cs].rearrange("c p n -> p c n"))
    nc.gpsimd.dma_start(out=wv_sb, in_=d_wv.rearrange("c p n -> p c n"))

    def dr_accum(psum, lhs_w, rhs_x):
        for cp in range(FC // 2):
            nc.tensor.matmul(
                psum,
                lhsT=lhs_w(slice(2 * cp, 2 * cp + 2)),
                rhs=rhs_x(slice(2 * cp, 2 * cp + 2)),
                start=(cp == 0), stop=(cp == FC // 2 - 1), perf_mode=DR)

    def swi_accum(psum, w_sb, blk, rhs_x):
        # weight-stationary fp8 pairs; host-packed [P, cp, 2, H] (DR) or
        # interleaved [P, cp, 2H] (SwI)
        for cp in range(FC // 2):
            if USE_SWI:
                lhsT = w_sb[:, cp, blk * 256:(blk + 1) * 256]
            else:
                w3 = w_sb.rearrange("p c (i n) -> p c i n", i=2)
                lhsT = w3[:, cp, :, blk * P:(blk + 1) * P]
            nc.tensor.matmul(
                psum,
                lhsT=lhsT,
                rhs=rhs_x(slice(2 * cp, 2 * cp + 2)),
                start=(cp == 0), stop=(cp == FC // 2 - 1),
                perf_mode=SWI if USE_SWI else DR)

    def unit_q(hp):
        qps = ps_mm.tile([P, SQ], F32, name="qps", tag="mm")
        swi_accum(qps, wq_sb, hp, lambda cs: xT_sb[:, cs, 0:SQ])
        nc.vector.tensor_copy(qT_all[:, hp, :], qps)

    def unit_k(hp, w):
        kps = ps_mm.tile([P, 512], F32, name="kps", tag="mm")
        swi_accum(kps, wk_sb, hp,
                  lambda cs: xT_sb[:, cs, w * 512:(w + 1) * 512])
        nc.vector.tensor_copy(kT_all[:, hp, w * 512:(w + 1) * 512], kps)

    def unit_v(g, t):
        vps = ps_mm.tile([P, 512], F32, name="vps", tag="mm")
        dr_accum(vps,
                 lambda cs: xT_sb[:, cs, t * P:(t + 1) * P],
                 lambda cs: wv_sb[:, cs, g * 512:(g + 1) * 512])
        nc.vector.tensor_copy(v_g[g][:, t, :, 0:HD],
                              vps.rearrange("p (h d) -> p h d", d=HD))

    unit_q(0)
    for w in range(NW):
        unit_k(0, w)
    for t in range(KC):
        unit_v(0, t)

    units = deque()
    vg1 = deque((1, t) for t in range(KC))
    for hp in range(1, FC):
        units.append(("q", hp, 0))
        for w in range(NW):
            units.append(("k", hp, w))
        for _ in range(3):
            if vg1:
                units.append(("v", *vg1.popleft()))
    while vg1:
        units.append(("v", *vg1.popleft()))

    def emit_unit():
        if not units:
            return
        kind, a, b2 = units.popleft()
        if kind == "q":
            unit_q(a)
        elif kind == "k":
            unit_k(a, b2)
        else:
            unit_v(a, b2)

    # ---------------- attention (head pairs) ----------------
    late = {}
    for hp in range(FC):
        if hp == 4:
            assert not units
            proj_ctx.close()
            late_pool = ctx.enter_context(tc.tile_pool(name="late", bufs=1))
            late["wd"] = late_pool.tile([P, FC, H], F8, name="wd_sb")
            late["rec"] = late_pool.tile([NH, SQ], BF16, name="rec_all")
            late["sel"] = late_pool.tile([NH, FC, P], BF16, name="sel")
            nc.gpsimd.dma_start(out=late["sel"], in_=d_sel)
            nc.gpsimd.dma_start(out=late["wd"],
                                in_=d_wd.rearrange("c p n -> p c n"))
        if hp == 5:
            # heads 0-7 are drained; normalize their ctx while attention runs
            nc.vector.reciprocal(late["rec"][0:8, :], sums2[0][:, :])
            for cc in range(FC // 2):
                bc_ps = ps_mm.tile([P, SQ], F32, name="bc_ps", tag="mm")
                nc.tensor.matmul(bc_ps, lhsT=late["sel"][0:8, cc, :],
                                 rhs=late["rec"][0:8, :], start=True, stop=True)
                nc.vector.tensor_mul(ctxT8[:, cc, :], ctxT_sb[:, cc, :], bc_ps)
        cps = [ps_c.tile([HD + 2, SQ], F32, name="ctx_ps") for _ in range(2)]
        for t in range(KC):
            s_ps = ps_s.tile([P, 2, SQ], F32, name="s_ps")
            mask_on_dve = t in MDVE_TS
            for h01 in range(2):
                rb = h01 * HD
                if not mask_on_dve:
                    nc.tensor.matmul(s_ps[:, h01, :], lhsT=identDD,
                                     rhs=maskT_sb[:, t, :],
                                     start=True, stop=False)
                nc.tensor.matmul(s_ps[:, h01, :],
                                 lhsT=kT_all[rb:rb + HD, hp, t * P:(t + 1) * P],
                                 rhs=qT_all[rb:rb + HD, hp, :],
                                 start=mask_on_dve, stop=True)
            emit_unit()
            pslot = pT_all[:, :, t % 4, :]
            if t in DVE_TS:
                nc.vector.tensor_scalar(pslot, s_ps, A_DVE, B_DVE,
                                        mybir.AluOpType.mult,
                                        mybir.AluOpType.add)
            else:
                nc.scalar.activation(pslot.bitcast(F8), s_ps,
                                     mybir.ActivationFunctionType.Exp,
                                     scale=EXP_SCALE)
            if mask_on_dve:
                nc.vector.tensor_tensor(pslot.bitcast(F8), pslot.bitcast(F8),
                                        _pair0(mask1_sb[:, t, :]),
                                        mybir.AluOpType.mult)
            if t % 2 == 1:
                for h01 in range(2):
                    h = 2 * hp + h01
                    nc.tensor.matmul(
                        cps[h01],
                        lhsT=v_g[h // 8][:, t - 1:t + 1, h % 8, :],
                        rhs=pT_all[:, h01, (t - 1) % 4:(t - 1) % 4 + 2, :].bitcast(F8),
                        start=(t == 1), stop=(t == KC - 1), perf_mode=DR)
        for h01 in range(2):
            h = 2 * hp + h01
            nc.vector.tensor_copy(
                ctxT_sb[h01 * HD:(h01 + 1) * HD, hp, :], cps[h01][0:HD, :])
            stmp = work.tile([1, SQ], F32, name="stmp")
            nc.vector.tensor_copy(stmp, cps[h01][HD:HD + 1, :])
            nc.sync.dma_start(out=sums2[h // 8][h % 8:h % 8 + 1, :], in_=stmp)

    # batched normalization -> fp8 ctx (second half; first half was hoisted)
    wd_sb = late["wd"]
    rec_all = late["rec"]
    nc.vector.reciprocal(rec_all[0:8, :], sums2[1][:, :])
    for cc in range(FC // 2, FC):
        bc_ps = ps_mm.tile([P, SQ], F32, name="bc_ps", tag="mm")
        nc.tensor.matmul(bc_ps, lhsT=late["sel"][0:8, cc, :],
                         rhs=rec_all[0:8, :], start=True, stop=True)
        nc.vector.tensor_mul(ctxT8[:, cc, :], ctxT_sb[:, cc, :], bc_ps)

    # ---------------- dense + residual + LayerNorm ----------------
    ln_pool = ctx.enter_context(tc.tile_pool(name="ln", bufs=2))
    gb_pool = ctx.enter_context(tc.tile_pool(name="gb", bufs=1))
    eps_t = gb_pool.tile([P, 1], F32)
    nc.vector.memset(eps_t, EPS)
    if affine:
        gamma_bc = gb_pool.tile([P, H], F32)
        beta_bc = gb_pool.tile([P, H], F32)
        nc.sync.dma_start(out=gamma_bc, in_=_bcast_ap(d_gamma, P))
        nc.sync.dma_start(out=beta_bc, in_=_bcast_ap(d_beta, P))

    for r in range(SQ // P):
        pre = ln_pool.tile([P, H], F32, name="pre")
        xres_t = ln_pool.tile([P, H], F32, name="xres_t")
        nc.sync.dma_start(out=xres_t, in_=d_xres[r])
        for nh2 in range(H // 512):
            dps = ps_mm.tile([P, 512], F32, name="dps", tag="mm")
            # dense: lhsT = ctx chunks (on-device, plain DR pairs); wd moving
            dr_accum(dps,
                     lambda cs: ctxT8[:, cs, r * P:(r + 1) * P],
                     lambda cs: wd_sb[:, cs, nh2 * 512:(nh2 + 1) * 512])
            nc.vector.tensor_add(pre[:, nh2 * 512:(nh2 + 1) * 512], dps,
                                 xres_t[:, nh2 * 512:(nh2 + 1) * 512])

        stats = ln_pool.tile([P, 2, 6], F32, name="stats")
        nc.vector.bn_stats(stats[:, 0, :], pre[:, 0:512])
        nc.vector.bn_stats(stats[:, 1, :], pre[:, 512:1024])
        mv = ln_pool.tile([P, 2], F32, name="mv")
        nc.vector.bn_aggr(mv, stats)
        std = ln_pool.tile([P, 1], F32, name="std")
        nc.scalar.activation(std, mv[:, 1:2], mybir.ActivationFunctionType.Sqrt,
                             bias=eps_t)
        rstd = ln_pool.tile([P, 1], F32, name="rstd")
        nc.vector.reciprocal(rstd, std)
        outv = ln_pool.tile([P, H], F32, name="outv")
        nc.vector.tensor_scalar(outv, pre, mv[:, 0:1], rstd,
                                mybir.AluOpType.subtract, mybir.AluOpType.mult)
        if affine:
            nc.vector.tensor_mul(outv, outv, gamma_bc)
            nc.vector.tensor_add(outv, outv, beta_bc)
        nc.sync.dma_start(out=d_out[r], in_=outv)


_nc_cache = {}


def _get_nc(affine):
    if affine not in _nc_cache:
        _nc_cache[affine] = _build_program(affine)
    return _nc_cache[affine]


def _to_f8(x):
    return np.clip(x, -240.0, 240.0).astype(NP_F8)


def kernel(hidden_states, mask, Wq, Wk, Wv, Wd, bd, gamma, beta):
    global last_results
    hidden_states = np.asarray(hidden_states, dtype=np.float32)
    mask = np.asarray(mask)
    Wq = np.asarray(Wq, dtype=np.float32)
    Wk = np.asarray(Wk, dtype=np.float32)
    Wv = np.asarray(Wv, dtype=np.float32)
    Wd = np.asarray(Wd, dtype=np.float32)
    bd = np.asarray(bd, dtype=np.float32)
    gamma = np.asarray(gamma, dtype=np.float32)
    beta = np.asarray(beta, dtype=np.float32)

    affine = bool(np.any(gamma != 1.0) or np.any(beta != 0.0))
    nc = _get_nc(affine)

    sel_np = np.zeros((NH, FC, P), dtype=np.float32)
    for cc in range(FC):
        sel_np[(2 * cc) % 8, cc, 0:HD] = 1.0
        sel_np[(2 * cc + 1) % 8, cc, HD:P] = 1.0
    sel_np = sel_np.astype(NP_BF16)

    def _swi_pack(W):
        # buf[k, cp, blk*256 + 2j + i] = W.T[(2cp+i)*128 + k, blk*128 + 127-j]
        wt = np.clip(WSCALE * W.T, -240.0, 240.0)
        a = wt.reshape(FC // 2, 2, P, FC, P)          # [cp, i, k, blk, m]
        rev = a[:, :, :, :, ::-1]                      # m -> 127-j
        swi = np.transpose(rev, (2, 0, 3, 4, 1))       # [k, cp, blk, j, i]
        swi = np.ascontiguousarray(swi).reshape(P, FC // 2, 2 * H)
        if not USE_SWI:
            # DR-pair layout: buf[cp, k, i*H + blk*128 + m] = wt[(2cp+i)*128+k, blk*128+m]
            b = np.transpose(a, (0, 2, 1, 3, 4))  # [cp, k, i, blk, m]
            return np.ascontiguousarray(
                b.astype(NP_F8).reshape(FC // 2, P, 2 * H))
        return np.ascontiguousarray(
            np.transpose(swi.astype(NP_F8), (1, 0, 2)))  # [cp, k, 2H]

    wqT = _swi_pack(Wq)
    wkT = _swi_pack(Wk)
    wvT = _to_f8(WSCALE * Wv.T).reshape(FC, P, H)
    wdT = _to_f8(WSCALE * Wd.T).reshape(FC, P, H)
    identDD = _to_f8(D_MASK * np.eye(P, dtype=np.float32))

    in_maps = []
    for c in range(NCORES):
        b, qi = c // 4, c % 4
        qs = qi * SQ
        xT = np.roll(hidden_states[b].T, -qs, axis=1)
        xT = _to_f8(np.ascontiguousarray(xT)).reshape(FC, P, S)
        maskR = np.roll(mask[b].T, -qs, axis=0)[:, qs:qs + SQ]
        maskT = _to_f8((maskR - 1.0) * D_MASK).reshape(KC, P, SQ)
        mask1 = _to_f8(maskR.astype(np.float32)).reshape(KC, P, SQ)
        xres = 1024.0 * (hidden_states[b, qs:qs + SQ] + bd[None, :])
        in_maps.append({
            "xT": xT,
            "wqT": wqT,
            "wkT": wkT,
            "wvT": wvT,
            "wdT": wdT,
            "maskT": maskT,
            "mask1": mask1,
            "identDD": identDD,
            "xres": np.ascontiguousarray(
                xres.astype(np.float32).reshape(SQ // P, P, H)),
            "gamma": gamma,
            "beta": beta,
            "sel": sel_np,
        })

    trace = os.environ.get("BASS_KERNEL_TRACE", "0") == "1"
    res = run_bass_kernel_spmd(
        nc, in_maps, core_ids=list(range(NCORES)), trace=trace
    )
    last_results = res

    out = np.empty((B, S, H), dtype=np.float32)
    for c in range(NCORES):
        b, qi = c // 4, c % 4
        out[b, qi * SQ:(qi + 1) * SQ] = res.results[c]["out"].reshape(SQ, H)
    return out
